# revision 16
# baseline (speedup 1.0000x reference)
"""Tropical (min-plus) matmul kernel for Trainium2, SPMD over 8 NeuronCores.

Computes out[b, j] = min_i (X[b, i] + W[j, i]) with B=1024, IN=OUT=512, fp32.

Sharding: data-parallel over batch — core c handles X rows [128c, 128(c+1)),
W replicated (matches the 1MB-weight replication hint).

Per-core pipeline (raw Bass, explicit semaphores):
  PE   : for each i, a K=3 matmul of an all-ones [3,128] stationary operand
         against the 3 bf16 limbs of W^T row i reconstructs W[j,i] broadcast
         over all 128 batch partitions into a PSUM bank — exact fp32 (the
         limbs sum exactly to the fp32 weight).
  ACT  : s_i = psum_bank + X[:, i] (Identity activation with per-partition
         bias; one fp32 round-to-nearest add, bit-matching the reference).
  DVE  : acc = min(acc, s_i) — in-place tensor_tensor min chain.
The three engines run as a software pipeline over an 8-bank PSUM ring and a
16-slot SBUF ring. Hardware allows at most one attached wait per compute
instruction and none on in-place ops, so in-place consumers use standalone
engine waits.
"""

import numpy as np
import ml_dtypes

import concourse.bass as bass
import concourse.mybir as mybir
from concourse.bass_utils import run_bass_kernel_spmd

B, IN, OUT = 1024, 512, 512
NCORES = 8
BLOC = B // NCORES  # batch rows per core = 128
IB = 8  # W^T limb chunks
IR = IN // IB  # i's per chunk = 64
SRING = 16  # SBUF s-tile ring slots
ACC_INIT = 1.0e30

_PROGRAM = None


def _build_program():
    nc = bass.Bass()
    x_in = nc.declare_dram_parameter("Xc", [BLOC, IN], mybir.dt.float32, isOutput=False)
    wtl_in = nc.declare_dram_parameter(
        "WTL", [3 * IB, IR * OUT], mybir.dt.bfloat16, isOutput=False
    )
    out_t = nc.declare_dram_parameter("OUTC", [BLOC, OUT], mybir.dt.float32, isOutput=True)

    with (
        nc.sbuf_tensor([BLOC, IN], mybir.dt.float32) as x_sb,
        nc.sbuf_tensor([3, 2, IR * OUT], mybir.dt.bfloat16) as wc,
        nc.sbuf_tensor([3, BLOC], mybir.dt.bfloat16) as ones_sb,
        nc.sbuf_tensor([BLOC, SRING, OUT], mybir.dt.float32) as sring,
        nc.sbuf_tensor([BLOC, OUT], mybir.dt.float32) as acc,
        nc.psum_tensor([BLOC, 8, OUT], mybir.dt.float32) as banks,
        nc.semaphore("dma_sem") as dma_sem,
        nc.semaphore("out_sem") as out_sem,
        nc.semaphore("wdma_sem") as wdma_sem,
        nc.semaphore("init_sem") as init_sem,
        nc.semaphore("pe_sem") as pe_sem,
        nc.semaphore("act_sem") as act_sem,
        nc.semaphore("dve_sem") as dve_sem,
        nc.Block() as blk,
    ):

        @blk.sync
        def _(sync):
            sync.dma_start(out=x_sb[:], in_=x_in[:, :]).then_inc(dma_sem, 16)
            for g in range(IB):
                if g >= 1:
                    # serialize chunk DMAs: completions of one semaphore can
                    # reorder, so a waiter on 16*(g+1) must imply all earlier
                    # chunks landed
                    sync.wait_ge(wdma_sem, 16 * g)
                if g >= 2:
                    # slot g%2 is free once PE finished chunk g-2
                    sync.wait_ge(pe_sem, (g - 1) * IR)
                sync.dma_start(
                    out=wc[:, g % 2, :], in_=wtl_in[3 * g : 3 * g + 3, :]
                ).then_inc(wdma_sem, 16)
            sync.wait_ge(dve_sem, IN + 1)
            sync.dma_start(out=out_t[:, :], in_=acc[:]).then_inc(out_sem, 16)

        @blk.vector
        def _(vector):
            nc.vector.memset(ones_sb[:], 1.0).then_inc(init_sem, 1)
            # dve_sem counts: 1 (acc memset) + one per stt. The self-wait per
            # iteration orders each in-place stt after the previous write's
            # completion (engines don't guarantee write visibility by program
            # order alone).
            nc.vector.memset(acc[:], ACC_INIT).then_inc(dve_sem, 1)
            vector.wait_ge(dma_sem, 16)  # x_sb loaded (scalar operand)
            for i in range(IN):
                # sring slot i was filled by ACT bulk-copy i//4
                vector.wait_ge(act_sem, i // 4 + 1)
                vector.wait_ge(dve_sem, i + 1)
                # acc = min(acc, w_bcast + X[:, i])
                nc.vector.scalar_tensor_tensor(
                    out=acc[:],
                    in0=sring[:, i % SRING, :],
                    scalar=x_sb[:, i : i + 1],
                    in1=acc[:],
                    op0=mybir.AluOpType.add,
                    op1=mybir.AluOpType.min,
                ).then_inc(dve_sem, 1)

        @blk.scalar
        def _(scalar):
            # Bulk-evict PSUM banks to the SBUF ring, 4 banks per ACTIVATE
            # (pure copy — the X-add is fused into the DVE stt).
            for k in range(IN // 4):
                i0 = 4 * k
                if i0 >= SRING:
                    # s-ring slots for [i0, i0+4) free once DVE consumed i0-1
                    # - (SRING-4) ... conservative: DVE done through
                    # i0 + 4 - SRING (+1 for the memset tick)
                    scalar.wait_ge(dve_sem, i0 + 4 - SRING + 1)
                ins = nc.scalar.copy(
                    sring[:, i0 % SRING : i0 % SRING + 4, :],
                    banks[:, i0 % 8 : i0 % 8 + 4, :],
                )
                ins._wait_ge(pe_sem, i0 + 4)
                ins.then_inc(act_sem, 1)

        @blk.tensor
        def _(tensor):
            tensor.wait_ge(init_sem, 1)  # ones memset
            for g in range(IB):
                tensor.wait_ge(wdma_sem, 16 * (g + 1))
                for r in range(IR):
                    i = g * IR + r
                    ins = nc.tensor.matmul(
                        banks[:, i % 8, :],
                        ones_sb[:],
                        wc[:, g % 2, r * OUT : (r + 1) * OUT],
                        start=True,
                        stop=True,
                    )
                    if i >= 8:
                        # bank group reused once the ACT copy covering i-8
                        # completed (ACT op k covers banks [4k, 4k+4))
                        ins._wait_ge(act_sem, (i - 8) // 4 + 1)
                    ins.then_inc(pe_sem, 1)

    return nc


def _w_limbs(W: np.ndarray) -> np.ndarray:
    """Split W^T into 3 bf16 limbs (exact fp32 reconstruction), laid out as
    [3*IB partitions, IR*OUT] so partition 3g+c holds limb c of i-block g."""
    WT = np.ascontiguousarray(W.T.astype(np.float32))  # [IN, OUT] = [i, j]
    l0 = WT.astype(ml_dtypes.bfloat16)
    r1 = WT - l0.astype(np.float32)
    l1 = r1.astype(ml_dtypes.bfloat16)
    r2 = r1 - l1.astype(np.float32)
    l2 = r2.astype(ml_dtypes.bfloat16)
    wtl = np.zeros((3 * IB, IR * OUT), dtype=ml_dtypes.bfloat16)
    for g in range(IB):
        blk = slice(g * IR, (g + 1) * IR)
        wtl[3 * g + 0, :] = l0[blk, :].reshape(-1)
        wtl[3 * g + 1, :] = l1[blk, :].reshape(-1)
        wtl[3 * g + 2, :] = l2[blk, :].reshape(-1)
    return wtl


def _run(X: np.ndarray, W: np.ndarray, trace: bool = False, **kwargs):
    global _PROGRAM
    X = np.asarray(X, dtype=np.float32)
    W = np.asarray(W, dtype=np.float32)
    assert X.shape == (B, IN) and W.shape == (OUT, IN)

    if _PROGRAM is None:
        _PROGRAM = _build_program()

    wtl = _w_limbs(W)
    in_maps = [
        {"Xc": np.ascontiguousarray(X[c * BLOC : (c + 1) * BLOC]), "WTL": wtl}
        for c in range(NCORES)
    ]
    res = run_bass_kernel_spmd(
        _PROGRAM, in_maps, list(range(NCORES)), trace=trace, **kwargs
    )
    out = np.concatenate([res.results[c]["OUTC"] for c in range(NCORES)], axis=0)
    return out.astype(np.float32), res


def kernel(X: np.ndarray, W: np.ndarray) -> np.ndarray:
    return _run(X, W)[0]


# revision 17
# speedup vs baseline: 1.1847x; 1.1847x over previous
"""Tropical (min-plus) matmul kernel for Trainium2, SPMD over 8 NeuronCores.

Computes out[b, j] = min_i (X[b, i] + W[j, i]) with B=1024, IN=OUT=512, fp32.

Sharding: data-parallel over batch — core c handles X rows [128c, 128(c+1)),
W replicated (matches the 1MB-weight replication hint).

Per-core pipeline (raw Bass, explicit semaphores):
  PE   : for each i, a K=3 matmul of an all-ones [3,128] stationary operand
         against the 3 bf16 limbs of W^T row i reconstructs W[j,i] broadcast
         over all 128 batch partitions into a PSUM bank — exact fp32 (the
         limbs sum exactly to the fp32 weight).
  ACT  : s_i = psum_bank + X[:, i] (Identity activation with per-partition
         bias; one fp32 round-to-nearest add, bit-matching the reference).
  DVE  : acc = min(acc, s_i) — in-place tensor_tensor min chain.
The three engines run as a software pipeline over an 8-bank PSUM ring and a
16-slot SBUF ring. Hardware allows at most one attached wait per compute
instruction and none on in-place ops, so in-place consumers use standalone
engine waits.
"""

import numpy as np
import ml_dtypes

import concourse.bass as bass
import concourse.mybir as mybir
from concourse.bass_utils import run_bass_kernel_spmd

B, IN, OUT = 1024, 512, 512
NCORES = 8
BLOC = B // NCORES  # batch rows per core = 128
IB = 8  # W^T limb chunks
IR = IN // IB  # i's per chunk = 64
SRING = 16  # SBUF s-tile ring slots
ACC_INIT = 1.0e30

_PROGRAM = None


def _build_program():
    nc = bass.Bass()
    x_in = nc.declare_dram_parameter("Xc", [BLOC, IN], mybir.dt.float32, isOutput=False)
    wtl_in = nc.declare_dram_parameter(
        "WTL", [3 * IB, IR * OUT], mybir.dt.bfloat16, isOutput=False
    )
    out_t = nc.declare_dram_parameter("OUTC", [BLOC, OUT], mybir.dt.float32, isOutput=True)

    with (
        nc.sbuf_tensor([BLOC, IN], mybir.dt.float32) as x_sb,
        nc.sbuf_tensor([3, 2, IR * OUT], mybir.dt.bfloat16) as wc,
        nc.sbuf_tensor([3, BLOC], mybir.dt.bfloat16) as ones_sb,
        nc.sbuf_tensor([BLOC, SRING, OUT], mybir.dt.float32) as sring,
        nc.sbuf_tensor([BLOC, OUT], mybir.dt.float32) as acc,
        nc.psum_tensor([BLOC, 8, OUT], mybir.dt.float32) as banks,
        nc.semaphore("dma_sem") as dma_sem,
        nc.semaphore("out_sem") as out_sem,
        nc.semaphore("wdma_sem") as wdma_sem,
        nc.semaphore("init_sem") as init_sem,
        nc.semaphore("pe_sem") as pe_sem,
        nc.semaphore("act_sem") as act_sem,
        nc.semaphore("dve_sem") as dve_sem,
        nc.Block() as blk,
    ):

        @blk.sync
        def _(sync):
            sync.dma_start(out=x_sb[:], in_=x_in[:, :]).then_inc(dma_sem, 16)
            for g in range(IB):
                if g >= 1:
                    # serialize chunk DMAs: completions of one semaphore can
                    # reorder, so a waiter on 16*(g+1) must imply all earlier
                    # chunks landed
                    sync.wait_ge(wdma_sem, 16 * g)
                if g >= 2:
                    # slot g%2 is free once PE finished chunk g-2
                    sync.wait_ge(pe_sem, (g - 1) * IR)
                sync.dma_start(
                    out=wc[:, g % 2, :], in_=wtl_in[3 * g : 3 * g + 3, :]
                ).then_inc(wdma_sem, 16)
            sync.wait_ge(dve_sem, IN + 1)
            sync.dma_start(out=out_t[:, :], in_=acc[:]).then_inc(out_sem, 16)

        @blk.vector
        def _(vector):
            nc.vector.memset(ones_sb[:], 1.0).then_inc(init_sem, 1)
            # dve_sem counts: 1 (acc memset) + one per stt. The self-wait per
            # iteration orders each in-place stt after the previous write's
            # completion (engines don't guarantee write visibility by program
            # order alone).
            nc.vector.memset(acc[:], ACC_INIT).then_inc(dve_sem, 1)
            for i in range(IN):
                vector.wait_ge(act_sem, i + 1)
                vector.wait_ge(dve_sem, i + 1)
                nc.vector.tensor_tensor(
                    acc[:], acc[:], sring[:, i % SRING, :], mybir.AluOpType.min
                ).then_inc(dve_sem, 1)

        @blk.scalar
        def _(scalar):
            scalar.wait_ge(dma_sem, 16)  # x_sb loaded (bias reads)
            for i in range(IN):
                if i >= SRING and i % 8 == 0:
                    # s-ring slots for [i, i+8) free once DVE consumed i-9
                    # (+1: dve_sem also counts the acc memset)
                    scalar.wait_ge(dve_sem, i + 8 - SRING + 1)
                ins = nc.scalar.activation(
                    sring[:, i % SRING, :],
                    banks[:, i % 8, :],
                    mybir.ActivationFunctionType.Identity,
                    bias=x_sb[:, i : i + 1],
                    scale=1.0,
                )
                ins._wait_ge(pe_sem, i + 1)
                ins.then_inc(act_sem, 1)

        @blk.tensor
        def _(tensor):
            tensor.wait_ge(init_sem, 1)  # ones memset
            for g in range(IB):
                tensor.wait_ge(wdma_sem, 16 * (g + 1))
                for r in range(IR):
                    i = g * IR + r
                    ins = nc.tensor.matmul(
                        banks[:, i % 8, :],
                        ones_sb[:],
                        wc[:, g % 2, r * OUT : (r + 1) * OUT],
                        start=True,
                        stop=True,
                    )
                    if i >= 8:
                        # bank slot reused once ACT consumed i-8
                        ins._wait_ge(act_sem, i - 7)
                    ins.then_inc(pe_sem, 1)

    return nc


def _w_limbs(W: np.ndarray) -> np.ndarray:
    """Split W^T into 3 bf16 limbs (exact fp32 reconstruction), laid out as
    [3*IB partitions, IR*OUT] so partition 3g+c holds limb c of i-block g."""
    WT = np.ascontiguousarray(W.T.astype(np.float32))  # [IN, OUT] = [i, j]
    l0 = WT.astype(ml_dtypes.bfloat16)
    r1 = WT - l0.astype(np.float32)
    l1 = r1.astype(ml_dtypes.bfloat16)
    r2 = r1 - l1.astype(np.float32)
    l2 = r2.astype(ml_dtypes.bfloat16)
    wtl = np.zeros((3 * IB, IR * OUT), dtype=ml_dtypes.bfloat16)
    for g in range(IB):
        blk = slice(g * IR, (g + 1) * IR)
        wtl[3 * g + 0, :] = l0[blk, :].reshape(-1)
        wtl[3 * g + 1, :] = l1[blk, :].reshape(-1)
        wtl[3 * g + 2, :] = l2[blk, :].reshape(-1)
    return wtl


def _run(X: np.ndarray, W: np.ndarray, trace: bool = False, **kwargs):
    global _PROGRAM
    X = np.asarray(X, dtype=np.float32)
    W = np.asarray(W, dtype=np.float32)
    assert X.shape == (B, IN) and W.shape == (OUT, IN)

    if _PROGRAM is None:
        _PROGRAM = _build_program()

    wtl = _w_limbs(W)
    in_maps = [
        {"Xc": np.ascontiguousarray(X[c * BLOC : (c + 1) * BLOC]), "WTL": wtl}
        for c in range(NCORES)
    ]
    res = run_bass_kernel_spmd(
        _PROGRAM, in_maps, list(range(NCORES)), trace=trace, **kwargs
    )
    out = np.concatenate([res.results[c]["OUTC"] for c in range(NCORES)], axis=0)
    return out.astype(np.float32), res


def kernel(X: np.ndarray, W: np.ndarray) -> np.ndarray:
    return _run(X, W)[0]


# revision 18
# speedup vs baseline: 1.4157x; 1.1950x over previous
"""Tropical (min-plus) matmul kernel for Trainium2, SPMD over 8 NeuronCores.

Computes out[b, j] = min_i (X[b, i] + W[j, i]) with B=1024, IN=OUT=512, fp32.

Sharding: data-parallel over batch - core c handles X rows [128c, 128(c+1)),
W replicated (the 1MB weight is cheap to replicate, per the hint).

Per-core pipeline (raw Bass, explicit semaphores):
  PE  : one K=6 bf16 matmul per i computes S_i[b, j] = X[b, i] + W[j, i] for
        all (b, j) into a PSUM bank: rows are the 3 bf16 limbs of X^T column i
        (paired with all-ones rhs rows) plus 3 all-ones rows (paired with the
        3 bf16 limbs of W^T row i). The limbs reconstruct the fp32 values
        exactly; only the final X+W add rounds (~2 ulp vs the reference).
  ACT : bulk-evicts 4 PSUM banks per ACTIVATE(Copy) into an SBUF ring.
  DVE : 4 parallel accumulator lanes in one [128, 4*512] tile; one in-place
        TENSOR_TENSOR min per 4 s-tiles (contiguous, unit stride - amortizes
        the per-op overhead), then a 2-step min-tree across the lanes.
Hardware allows at most one attached semaphore wait per compute instruction
and none on in-place ops, so in-place consumers use standalone engine waits;
single-semaphore DMA chains are serialized (completions can reorder).
"""

import numpy as np
import ml_dtypes

import concourse.bass as bass
import concourse.mybir as mybir
from concourse.bass_utils import run_bass_kernel_spmd

B, IN, OUT = 1024, 512, 512
NCORES = 8
BLOC = B // NCORES  # 128
IB = 16  # chunks
IR = IN // IB  # 32 i's per chunk
SRING = 32  # SBUF s-tile ring slots
GROUP = 4  # i's per DVE tensor_tensor (4 parallel accumulators)
NGRP = IN // GROUP  # 128
ACC_INIT = 1.0e30

_PROGRAM = None


def _build_program():
    nc = bass.Bass()
    xl_in = nc.declare_dram_parameter(
        "XL6", [6 * IB, IR * BLOC], mybir.dt.bfloat16, isOutput=False
    )
    wtl_in = nc.declare_dram_parameter(
        "WTL6", [6 * IB, IR * OUT], mybir.dt.bfloat16, isOutput=False
    )
    out_t = nc.declare_dram_parameter("OUTC", [BLOC, OUT], mybir.dt.float32, isOutput=True)

    with (
        nc.sbuf_tensor([6, 2, IR * BLOC], mybir.dt.bfloat16) as xc,
        nc.sbuf_tensor([6, 2, IR * OUT], mybir.dt.bfloat16) as wc,
        nc.sbuf_tensor([BLOC, SRING, OUT], mybir.dt.float32) as sring,
        nc.sbuf_tensor([BLOC, GROUP, OUT], mybir.dt.float32) as acc,
        nc.psum_tensor([BLOC, 8, OUT], mybir.dt.float32) as banks,
        nc.semaphore("out_sem") as out_sem,
        nc.semaphore("wdma_sem") as wdma_sem,
        nc.semaphore("pe_sem") as pe_sem,
        nc.semaphore("act_sem") as act_sem,
        nc.semaphore("dve_sem") as dve_sem,
        nc.Block() as blk,
    ):

        @blk.sync
        def _(sync):
            for g in range(IB):
                # serialize the two chunk DMAs (sem completions can reorder)
                if g >= 1:
                    sync.wait_ge(wdma_sem, 32 * g)
                if g >= 2:
                    # slot g%2 free once PE finished chunk g-2
                    sync.wait_ge(pe_sem, (g - 1) * IR)
                sync.dma_start(
                    out=wc[:, g % 2, :], in_=wtl_in[6 * g : 6 * g + 6, :]
                ).then_inc(wdma_sem, 16)
                sync.wait_ge(wdma_sem, 32 * g + 16)
                sync.dma_start(
                    out=xc[:, g % 2, :], in_=xl_in[6 * g : 6 * g + 6, :]
                ).then_inc(wdma_sem, 16)
            sync.wait_ge(dve_sem, NGRP + 3)
            sync.dma_start(out=out_t[:, :], in_=acc[:, 0, :]).then_inc(out_sem, 16)

        @blk.vector
        def _(vector):
            # dve_sem ticks: 1 (acc memset), then one per group TT (group k
            # done at tick k+2), then two tree-combine ticks.
            nc.vector.memset(acc[:], ACC_INIT).then_inc(dve_sem, 1)
            for q in range(NGRP):
                s0 = (q * GROUP) % SRING
                vector.wait_ge(act_sem, q + 1)
                vector.wait_ge(dve_sem, q + 1)
                nc.vector.tensor_tensor(
                    acc[:],
                    acc[:],
                    sring[:, s0 : s0 + GROUP, :],
                    mybir.AluOpType.min,
                ).then_inc(dve_sem, 1)
            # min-tree across the 4 accumulator lanes
            vector.wait_ge(dve_sem, NGRP + 1)
            nc.vector.tensor_tensor(
                acc[:, 0:2, :], acc[:, 0:2, :], acc[:, 2:4, :], mybir.AluOpType.min
            ).then_inc(dve_sem, 1)
            vector.wait_ge(dve_sem, NGRP + 2)
            nc.vector.tensor_tensor(
                acc[:, 0, :], acc[:, 0, :], acc[:, 1, :], mybir.AluOpType.min
            ).then_inc(dve_sem, 1)

        @blk.scalar
        def _(scalar):
            # Bulk-evict PSUM banks to the SBUF ring, 4 banks per op.
            for m in range(IN // 4):
                if 4 * m >= SRING:
                    # slots reused once the DVE group TT covering them ran
                    scalar.wait_ge(dve_sem, m - SRING // 4 + 2)
                ins = nc.scalar.copy(
                    sring[:, (4 * m) % SRING : (4 * m) % SRING + 4, :],
                    banks[:, (4 * m) % 8 : (4 * m) % 8 + 4, :],
                )
                ins._wait_ge(pe_sem, 4 * m + 4)
                ins.then_inc(act_sem, 1)

        @blk.tensor
        def _(tensor):
            for g in range(IB):
                tensor.wait_ge(wdma_sem, 32 * (g + 1))
                for r in range(IR):
                    i = g * IR + r
                    ins = nc.tensor.matmul(
                        banks[:, i % 8, :],
                        xc[:, g % 2, r * BLOC : (r + 1) * BLOC],
                        wc[:, g % 2, r * OUT : (r + 1) * OUT],
                        start=True,
                        stop=True,
                    )
                    if i >= 8:
                        ins._wait_ge(act_sem, (i - 8) // 4 + 1)
                    ins.then_inc(pe_sem, 1)

    return nc


def _limbs3(A: np.ndarray):
    l0 = A.astype(ml_dtypes.bfloat16)
    r1 = A - l0.astype(np.float32)
    l1 = r1.astype(ml_dtypes.bfloat16)
    r2 = r1 - l1.astype(np.float32)
    l2 = r2.astype(ml_dtypes.bfloat16)
    return l0, l1, l2


def _pack6(T: np.ndarray, limb_rows_first: bool, ncols: int) -> np.ndarray:
    """Pack [IN, ncols] fp32 into [6*IB, IR*ncols] bf16: per chunk g, three
    limb rows and three ones rows (limbs first or last)."""
    ls = _limbs3(np.ascontiguousarray(T.astype(np.float32)))
    outp = np.ones((6 * IB, IR * ncols), dtype=ml_dtypes.bfloat16)
    for g in range(IB):
        blk = slice(g * IR, (g + 1) * IR)
        for c in range(3):
            row = 6 * g + c if limb_rows_first else 6 * g + 3 + c
            outp[row, :] = ls[c][blk, :].reshape(-1)
    return outp


def _run(X: np.ndarray, W: np.ndarray, trace: bool = False, **kwargs):
    global _PROGRAM
    X = np.asarray(X, dtype=np.float32)
    W = np.asarray(W, dtype=np.float32)
    assert X.shape == (B, IN) and W.shape == (OUT, IN)

    if _PROGRAM is None:
        _PROGRAM = _build_program()

    wtl6 = _pack6(W.T, limb_rows_first=False, ncols=OUT)  # rows 3-5 = W^T limbs
    in_maps = []
    for c in range(NCORES):
        xt = X[c * BLOC : (c + 1) * BLOC].T  # [IN, BLOC]
        xl6 = _pack6(xt, limb_rows_first=True, ncols=BLOC)  # rows 0-2 = X^T limbs
        in_maps.append({"XL6": xl6, "WTL6": wtl6})
    res = run_bass_kernel_spmd(
        _PROGRAM, in_maps, list(range(NCORES)), trace=trace, **kwargs
    )
    out = np.concatenate([res.results[c]["OUTC"] for c in range(NCORES)], axis=0)
    return out.astype(np.float32), res


def kernel(X: np.ndarray, W: np.ndarray) -> np.ndarray:
    return _run(X, W)[0]
